# revision 39
# baseline (speedup 1.0000x reference)
"""Trainium2 Bass kernel for nn_Attention_62706522521647.

Dense multi-head attention with QK-L2-norm (learnable scale) + axial RoPE,
B=4 N=2048 H=8 DQ=DV=48, IN_DIM=384, f32 inputs/outputs.

Sharding (8 cores, no collectives): core c handles batch b=c//2 and the
4 heads [4*(c%2), 4*(c%2)+4).  Each core computes a partial output
(its heads' contribution through the output projection); the host sums
the two partials per batch.

Per-core design (v2 — ACT-exp is the critical path, everything else hides
under it):
 - Engine APs start at partitions 0/32/64/96, so two heads are packed per
   [128, N] tile at rows 0-47 and 64-111 (pad rows zeroed via zero weight
   columns).
 - xT fed pre-transposed: xT [384, 2048] bf16 (3 chunks of [128, 2048]).
 - RoPE swap(q) via a SECOND projection with host-swapped weight columns;
   rotation is qr = raw*C2 + swp*S2 with signs baked into host-wrapped
   angles (HW Sin needs [-pi,pi]; cos = Sin(wrap(pi/2 - theta))).  Angle
   tables shipped fp16 (halves the DMA).
 - BOTH q and k are pre-normalized: rsq = exp(-0.5*ln(ssq/s + eps)) on ACT
   (ln+exp live in ONE table set with the attention exps -> only 2
   ACT_TABLE_LOADs in the whole kernel: trig + natural_log_exp).
   ssq for both q-halves packed into one psum tile via tile_position
   (0, 32*qh) so a single ln/exp pair covers the whole tensor.
 - scores TRANSPOSED: sT[k, q] = kn-chunk.T @ qn, 2 heads row-packed via
   tile_position (0,0)/(64,0); exp with immediate scale=1 (no per-partition
   operand); softmax denominator via ones column in the AV stationary
   [v|0*16|1] (M=65): Z lands in psum row 64.  No max-subtraction
   (scores in [-10,10]).
 - Attention inner loop is software-pipelined: scores(ch+1) are emitted
   BEFORE AV(ch) so the PE fills the gap while ACT streams exps; PSUM =
   2x scores bufs (4 banks) + 2x AV accumulators (4 banks).
 - Z reciprocal via DVE reciprocal_approx_fast (51 ULP, off the ACT path);
   normalize of block b is emitted inside block b+1 (hook at ch==3) so its
   PE/DVE work hides under the exp stream.  Out-projection chunks 0-7 are
   emitted inside the last block (hook at ch==10); 8-15 at the tail.
"""

import math

import numpy as np
import ml_dtypes

B, N, H, DQ, DV = 4, 2048, 8, 48, 48
IN_DIM = H * DQ  # 384
D2 = DQ // 2  # 24
MAX_FREQ = 10.0
EPS = 1e-6
NCORES = 8
HPC = 4  # heads per core
KC = IN_DIM // 128  # 3 contraction chunks for projections
NCH = N // 128  # 16 k-chunks of 128
NQH = 2  # q halves of 1024
QW = 1024  # q tile width
BF16 = ml_dtypes.bfloat16
FP16 = np.float16


def _freqs_np():
    """Reference freqs (numpy f64 ~1e-7 from the jax f32 original — far
    inside the fp16 angle quantization already applied to the tables)."""
    log_min = math.log(math.pi)
    log_max = math.log(MAX_FREQ * math.pi)
    n = H * D2
    f = np.exp(np.linspace(log_min, log_max, n + 1)[:-1])
    return f.reshape(D2, H).T.astype(np.float32)  # [H, 24]


def build_nc(inv_scale: float):
    import concourse.bass as bass
    import concourse.tile as tile
    from concourse import bacc, mybir

    from concourse.alu_op_type import AluOpType

    dt = mybir.dt
    AF = mybir.ActivationFunctionType
    F32, B16, F16 = dt.float32, dt.bfloat16, dt.float16

    nc = bacc.Bacc("TRN2")

    xT = nc.dram_tensor("xT", [KC, 128, N], B16, kind="ExternalInput")
    tcd = nc.dram_tensor("tcd", [2, 128, N], F16, kind="ExternalInput")
    tsd = nc.dram_tensor("tsd", [2, 128, N], F16, kind="ExternalInput")
    # q/k weights: per pack 112 cols (headA 0-47, zeros 48-63, headB 64-111)
    wq = nc.dram_tensor("wq", [KC, 128, 224], B16, kind="ExternalInput")
    wqs = nc.dram_tensor("wqs", [KC, 128, 224], B16, kind="ExternalInput")
    wk = nc.dram_tensor("wk", [KC, 128, 224], B16, kind="ExternalInput")
    wks = nc.dram_tensor("wks", [KC, 128, 224], B16, kind="ExternalInput")
    wv = nc.dram_tensor("wv", [KC, 128, 192], B16, kind="ExternalInput")
    wo = nc.dram_tensor("wo", [2, 128, 384], B16, kind="ExternalInput")
    e2d = nc.dram_tensor("e2d", [2, 112], F32, kind="ExternalInput")
    out = nc.dram_tensor("out", [N, IN_DIM], F16, kind="ExternalOutput")

    with tile.TileContext(nc) as tc:
        with (
            tc.tile_pool(name="consts", bufs=1) as consts,
            tc.tile_pool(name="trig", bufs=2) as trig,
            tc.tile_pool(name="qk", bufs=1) as qkpool,
            tc.tile_pool(name="work", bufs=2) as work,
            tc.tile_pool(name="esb", bufs=3) as esb,
            tc.tile_pool(name="psS", bufs=2, space=bass.MemorySpace.PSUM) as psS,
            tc.tile_pool(name="psO", bufs=2, space=bass.MemorySpace.PSUM) as psO,
        ):
            # ------------- input DMAs: merged, via HWDGE (no gpsimd) -------
            # scalar ring FIRST: pack-0 trig feeds the longest prep chain
            # (sin -> rope -> norms); sync ring: compute inputs in
            # consumption order
            tc_all = consts.tile([128, 2, N], F16, tag="tc")
            ts_all = consts.tile([128, 2, N], F16, tag="ts")
            for p in range(2):
                nc.scalar.dma_start(out=tc_all[:, p, :], in_=tcd[p])
                nc.scalar.dma_start(out=ts_all[:, p, :], in_=tsd[p])
            th_sb = {p: (tc_all[:, p, :], ts_all[:, p, :]) for p in range(2)}
            xT_all = consts.tile([128, KC, N], B16, tag="xT")
            nc.sync.dma_start(out=xT_all, in_=xT.rearrange("k p n -> p k n"))
            xT_sb = [xT_all[:, kc, :] for kc in range(KC)]
            wv_all = consts.tile([128, KC, 192], B16, tag="wv")
            nc.sync.dma_start(out=wv_all, in_=wv.rearrange("k p m -> p k m"))
            wv_sb = [wv_all[:, kc, :] for kc in range(KC)]
            w_sb = {}
            for nm, hd in (("wq", wq), ("wqs", wqs), ("wk", wk), ("wks", wks)):
                t = consts.tile([128, KC, 224], B16, tag=nm, name=nm)
                nc.sync.dma_start(out=t, in_=hd.rearrange("k p m -> p k m"))
                for kc in range(KC):
                    w_sb[(nm, kc)] = t[:, kc, :]
            wo_all = consts.tile([128, 2, 384], B16, tag="wo")
            nc.sync.dma_start(out=wo_all, in_=wo.rearrange("k p m -> p k m"))
            wo_sb = [wo_all[:, p, :] for p in range(2)]
            # E2 replicated at rows 0-1 and 32-33 (matmul requires lhsT and
            # rhs at the same base partition; rsq for qh=1 sits at rows 32-33)
            E2 = consts.tile([34, 112], F32, tag="E2")
            nc.scalar.dma_start(out=E2[0:2, :], in_=e2d[:])
            nc.scalar.dma_start(out=E2[32:34, :], in_=e2d[:])

            # ---------------- constants / memsets (gpsimd: free engine) ---
            # ssq reduction stationaries, M=34 with zero pad cols so every
            # psum row of ps_ssq is written (race-detector/garbage safety):
            # qh0 sums land in rows 0-1 (ones2a), qh1 in rows 32-33 (ones2b)
            ones2a = consts.tile([128, 128], B16, tag="ones2a")
            nc.vector.memset(ones2a, 0.0)
            nc.vector.memset(ones2a[0:48, 0:1], 1.0)
            nc.vector.memset(ones2a[64:112, 1:2], 1.0)
            ones2b = consts.tile([128, 128], B16, tag="ones2b")
            nc.vector.memset(ones2b, 0.0)
            nc.vector.memset(ones2b[0:48, 32:33], 1.0)
            nc.vector.memset(ones2b[64:112, 33:34], 1.0)
            # Ez: z-broadcast stationary: row 64 -> cols 0-47, row 96 -> 64-111
            Ez = consts.tile([128, 112], F32, tag="Ez")
            nc.vector.memset(Ez, 0.0)
            nc.vector.memset(Ez[64:65, 0:48], 1.0)
            nc.vector.memset(Ez[96:97, 64:112], 1.0)
            # zq: Z staging, all-finite by construction (rows 64/96 overwritten)
            zq = work.tile([128, QW], F32, tag="zq", bufs=1)
            nc.gpsimd.memset(zq, 1.0)
            # activation-bias constants
            cdb = consts.tile([128, 2], F32, tag="cdb")
            for col, val in enumerate([0.0, EPS]):
                nc.vector.memset(cdb[:, col : col + 1], val)
                nc.const_aps.aps[(F32, val)] = cdb[:, col : col + 1]

            # v stationary per (chunk, head): [v(48) | zeros | one].  The
            # ones column sits at 64 for slot-0 heads and 96 for slot-1
            # heads, so Z lands at psum partition 64/96 respectively and the
            # 1/Z reciprocal into zq needs no partition shift (HW custom-DVE
            # ops don't shift partitions even though the sim models it).
            v4 = consts.tile([128, NCH, HPC, 97], B16, tag="v4")
            nc.gpsimd.memset(v4[:, :, :, 48:97], 0.0)
            for hp in range(HPC):
                zc = 64 if hp % 2 == 0 else 96
                nc.gpsimd.memset(v4[:, :, hp, zc : zc + 1], 1.0)

            # packed attention outputs (pad rows must be finite zeros for the
            # out-projection: garbage bf16 could be NaN and NaN*0 = NaN)
            on_pack = [
                qkpool.tile([128, N], B16, tag=f"on{p}", name=f"on{p}")
                for p in range(2)
            ]
            for p in range(2):
                # rows 48-63 / 112-127 must be finite zeros; bases limited to
                # 32-multiples, the extra rows are overwritten by normalize
                nc.gpsimd.memset(on_pack[p][32:64, :], 0.0)
                nc.gpsimd.memset(on_pack[p][96:128, :], 0.0)

            # ---------------- trig tables: Sin first (trig table set) ------
            C2, S2 = [], []
            for p in range(2):
                tct, tst = th_sb[p]
                c2t = trig.tile([128, N], B16, tag="c2t", name=f"c2t{p}", bufs=2)
                nc.scalar.activation(c2t, tct, AF.Sin)
                C2.append(c2t)
                s2t = trig.tile([128, N], B16, tag="s2t", name=f"s2t{p}", bufs=2)
                nc.scalar.activation(s2t, tst, AF.Sin)
                S2.append(s2t)
            dummy = work.tile([128, 1], F32, tag="dummy", bufs=1)

            def pe_warmup(n):
                # HAM un-throttles (K=4/8 -> 8/8, 1.2 -> 2.4 GHz) only after
                # ~3.4us of GAPLESS PE activity; dependency-free back-to-back
                # dummy matmuls provide it.  FIFO position controls WHEN.
                wps = psS.tile([128, 512], F32, tag="s", name="warm")
                for _ in range(n):
                    nc.tensor.matmul(
                        wps, xT_sb[0][:, 0:128], xT_sb[1][:, 0:512],
                        start=True, stop=True,
                    )

            # ---------------- v projection ----------------
            pe_warmup(9)
            for ch in range(NCH):
                pool, tg = (psS, "s") if ch % 2 == 0 else (psO, "o")
                ps_v = pool.tile([128, 192], F32, tag=tg, name="ps_v")
                for kc in range(KC):
                    nc.tensor.matmul(
                        ps_v,
                        xT_sb[kc][:, 128 * ch : 128 * (ch + 1)],
                        wv_sb[kc],
                        start=(kc == 0),
                        stop=(kc == KC - 1),
                    )
                nc.scalar.copy(
                    v4[:, ch, :, 0:48],
                    ps_v.rearrange("p (h d) -> p h d", h=HPC),
                )

            # ---------------- q/k projections + rope (PE + DVE) -----------
            qn = [
                qkpool.tile([128, N], B16, tag=f"qn{p}", name=f"qn{p}")
                for p in range(2)
            ]
            kn = [
                qkpool.tile([128, N], B16, tag=f"kn{p}", name=f"kn{p}")
                for p in range(2)
            ]
            qr_t = {}  # (p, name) -> rope'd (un-normalized) [128, N] bf16
            tensors = [(p, name) for p in range(2) for name in ("q", "k")]

            for p, name in tensors:
                c2t, s2t = C2[p], S2[p]
                qr = work.tile(
                    [128, N], B16, tag="qr", name=f"qr_{name}{p}", bufs=4
                )
                qr_t[(p, name)] = qr
                for nh in range(2):
                    ns = 1024 * nh
                    raw = psS.tile([112, 1024], F32, tag="s", name="raw")
                    swp = psO.tile([112, 1024], F32, tag="o", name="swp")
                    for half in range(2):
                        hs = 512 * half
                        for kc in range(KC):
                            nc.tensor.matmul(
                                raw[:, hs : hs + 512],
                                w_sb[("w" + name, kc)][:, 112 * p : 112 * (p + 1)],
                                xT_sb[kc][:, ns + hs : ns + hs + 512],
                                start=(kc == 0),
                                stop=(kc == KC - 1),
                            )
                        for kc in range(KC):
                            nc.tensor.matmul(
                                swp[:, hs : hs + 512],
                                w_sb[("w" + name + "s", kc)][:, 112 * p : 112 * (p + 1)],
                                xT_sb[kc][:, ns + hs : ns + hs + 512],
                                start=(kc == 0),
                                stop=(kc == KC - 1),
                            )
                    t1 = work.tile([112, 1024], B16, tag="t1", name="t1", bufs=2)
                    nc.vector.tensor_mul(t1, raw, c2t[0:112, ns : ns + 1024])
                    t2 = work.tile([112, 1024], B16, tag="t2", name="t2", bufs=2)
                    nc.vector.tensor_mul(t2, swp, s2t[0:112, ns : ns + 1024])
                    # the add runs on gpsimd (idle engine) to shorten the
                    # DVE prep pole; ~2.2us each but fully overlapped
                    nc.gpsimd.tensor_tensor(
                        qr[0:112, ns : ns + 1024], t1, t2, AluOpType.add
                    )

            # norms, wave-pipelined across the 4 tensors: ssq packed [34, QW]
            # (qh0 sums rows 0-1, qh1 rows 32-33); sqrt on ACT (single table
            # set), 1/sqrt via DVE reciprocal_approx_fast (keeps the ACT
            # stream to 3 table loads total: trig -> sqrt -> exp)
            sq_t, ssq_t, rsq_t = {}, {}, {}
            for key in tensors:
                qr = qr_t[key]
                sq = work.tile([112, N], B16, tag="sq", name="sq", bufs=2)
                nc.scalar.activation(sq, qr[0:112, :], AF.Square)
                sq_t[key] = sq
            for key in tensors:
                sq = sq_t[key]
                ps_ssq = psO.tile([128, QW], F32, tag="o", name="ps_ssq")
                for hh in range(2):
                    for qh in range(NQH):
                        ns = QW * qh + 512 * hh
                        nc.tensor.matmul(
                            ps_ssq[:, 512 * hh : 512 * (hh + 1)],
                            (ones2a if qh == 0 else ones2b)[0:112, :],
                            sq[:, ns : ns + 512],
                            start=(qh == 0),
                            stop=(qh == 1),
                        )
                ssq_t[key] = ps_ssq
            for key in tensors:
                sqq = work.tile([128, QW], F32, tag="sqq", name="sqq", bufs=2)
                nc.scalar.activation(
                    sqq, ssq_t[key], AF.Sqrt, scale=inv_scale, bias=EPS
                )
                rsq = work.tile([128, QW], F32, tag="rsq", name="rsq", bufs=4)
                nc.vector.reciprocal_approx_fast(out=rsq, in_=sqq)
                rsq_t[key] = rsq
            # prefetch the exp table set now (off the attention critical path)
            nc.scalar.activation(dummy, cdb[:, 0:1], AF.Exp)
            for i, (p, name) in enumerate(tensors):
                qr, rsq = qr_t[(p, name)], rsq_t[(p, name)]
                dst = qn[p] if name == "q" else kn[p]
                for qh in range(NQH):
                    pool, tg = ((psS, "s") if (2 * i + qh) % 2 == 0
                                else (psO, "o"))
                    ps_rb = pool.tile([112, QW], F32, tag=tg, name="ps_rb")
                    for hh in range(2):
                        nc.tensor.matmul(
                            ps_rb[:, 512 * hh : 512 * (hh + 1)],
                            E2[32 * qh : 32 * qh + 2, :],
                            rsq[32 * qh : 32 * qh + 2,
                                512 * hh : 512 * (hh + 1)],
                            start=True,
                            stop=True,
                        )
                    qs = QW * qh
                    nc.vector.tensor_mul(
                        dst[0:112, qs : qs + QW],
                        qr[0:112, qs : qs + QW],
                        ps_rb,
                    )

            # ---------------- attention ----------------
            row0 = {0: 0, 1: 64}  # head slot -> pack row offset

            def normalize_a(o0, o1, tail=False):
                """Block-end stage: free the AV psum banks ASAP.  1/Z via
                reciprocal_approx_fast straight off the Z rows of o_ps into
                zq rows 64/96 (zq is memset 1.0, so every row stays finite
                for the Ez broadcast); o values copied out to bf16."""
                cp = nc.scalar.copy if tail else nc.vector.tensor_copy
                obufs = []
                for i, o in enumerate((o0, o1)):
                    zr = 64 + 32 * i
                    cp(zq[zr : zr + 1, :], o[zr : zr + 1, :])
                    ob = work.tile([48, QW], B16, tag=f"ob{i}", name=f"ob{i}",
                                   bufs=2)
                    cp(ob, o[0:48, :])
                    obufs.append(ob)
                # full-tile SBUF->SBUF recip (the only form HW handles)
                rzb = work.tile([128, QW], F32, tag="rzb", name="rzb", bufs=2)
                nc.vector.reciprocal_approx_fast(out=rzb, in_=zq)
                return obufs, rzb

            def normalize_b(p, qh, obufs, rzb):
                """Next-block stage (PE + DVE, hidden under the exp stream):
                broadcast 1/Z over the head rows and scale."""
                qs = QW * qh
                ps_r = psS.tile([112, QW], F32, tag="s", name="ps_r")
                for hh in range(2):
                    nc.tensor.matmul(
                        ps_r[:, 512 * hh : 512 * (hh + 1)],
                        Ez,
                        rzb[:, 512 * hh : 512 * (hh + 1)],
                        start=True,
                        stop=True,
                    )
                for i in range(2):
                    r = row0[i]
                    nc.vector.tensor_mul(
                        on_pack[p][r : r + 48, qs : qs + QW],
                        obufs[i],
                        ps_r[r : r + 48, :],
                    )

            # output staged in SBUF, shipped in two big HWDGE DMAs
            osb_all = consts.tile([128, NCH, 384], F16, tag="osb")
            out_r = out.rearrange("(c p) m -> p c m", p=128)

            def outproj(chs, flush, tail=False):
                cp = nc.scalar.copy if tail else nc.vector.tensor_copy
                for ch in chs:
                    ns = 128 * ch
                    ps_out = psS.tile([128, 384], F32, tag="s", name="ps_out")
                    for p in range(2):
                        nc.tensor.matmul(
                            ps_out,
                            on_pack[p][:, ns : ns + 128],
                            wo_sb[p],
                            start=(p == 0),
                            stop=(p == 1),
                        )
                    cp(osb_all[:, ch, :], ps_out)
                nc.sync.dma_start(
                    out=out_r[:, flush[0] : flush[1], :],
                    in_=osb_all[:, flush[0] : flush[1], :],
                )

            def attn_block(p, qh, hooks, warm=0):
                qs = QW * qh
                o = [
                    psO.tile([65 + 32 * i, QW], F32, tag="o", name=f"o{i}")
                    for i in range(2)
                ]
                stiles = {}

                def emit_scores(ch):
                    ks = 128 * ch
                    for i in range(2):
                        r = row0[i]
                        s = psS.tile([128, QW], F32, tag="s", name=f"s{i}")
                        for hh in range(2):
                            nc.tensor.matmul(
                                s[:, 512 * hh : 512 * (hh + 1)],
                                kn[p][r : r + 48, ks : ks + 128],
                                qn[p][r : r + 48,
                                      qs + 512 * hh : qs + 512 * (hh + 1)],
                                start=True,
                                stop=True,
                                tile_position=(r, 0),
                            )
                        stiles[(ch, i)] = s

                if warm:
                    # FIFO slot right before the first scores: executes after
                    # the last prep matmul, flipping HAM to 8/8 for the
                    # attention stream (gap to first scores < 3.4us MID window)
                    pe_warmup(warm)
                emit_scores(0)
                for ch in range(NCH):
                    es = []
                    for i in range(2):
                        e = esb.tile([128, QW], B16, tag=f"e{i}", name=f"e{i}")
                        nc.scalar.activation(e, stiles.pop((ch, i)), AF.Exp)
                        es.append(e)
                    if ch + 1 < NCH:
                        emit_scores(ch + 1)
                    for i in range(2):
                        for hh in range(2):
                            nc.tensor.matmul(
                                o[i][:, 512 * hh : 512 * (hh + 1)],
                                v4[:, ch, 2 * p + i, 0 : 65 + 32 * i],
                                es[i][:, 512 * hh : 512 * (hh + 1)],
                                start=(ch == 0),
                                stop=(ch == NCH - 1),
                            )
                    if ch in (5, 11):
                        # periodic re-warm: HAM oscillates under the micro-
                        # idle pattern of the exp-bound stream (K18 failure
                        # mode); short gapless bursts keep K=8/8
                        pe_warmup(6)
                    if ch in hooks:
                        hooks[ch]()
                return o

            prev = None
            for p in range(2):
                for qh in range(NQH):
                    hooks = {}
                    if prev is not None:
                        pv = prev
                        hooks[3] = lambda pv=pv: normalize_b(*pv)
                    if (p, qh) == (1, 1):
                        hooks[10] = lambda: outproj(range(8), (0, 8))
                    o = attn_block(p, qh, hooks, warm=9 if prev is None else 0)
                    obufs, rzb = normalize_a(o[0], o[1], tail=(p, qh) == (1, 1))
                    prev = (p, qh, obufs, rzb)
            normalize_b(*prev)
            outproj(range(8, 16), (8, 16), tail=True)

    return nc


def make_in_maps(x, pos, Wq, Wkv, Wout, scale):
    """Build the 8 per-core input dicts (host-side sharding + layout)."""
    freqs = _freqs_np()  # [H, 24]
    sroot = np.sqrt(scale.astype(np.float64))  # [H]
    in_maps = []
    for c in range(NCORES):
        b = c // 2
        hb = HPC * (c % 2)
        heads = list(range(hb, hb + HPC))
        xb = x[b].astype(np.float32)  # [N, 384]
        xT = np.ascontiguousarray(xb.T).reshape(KC, 128, N)
        posT = np.ascontiguousarray(pos[b].T).astype(np.float32)  # [24, N]

        def wrap(a):  # -> [-pi, pi], in f64 then back
            return (np.mod(a.astype(np.float64) + np.pi, 2 * np.pi) - np.pi)

        tcd = np.zeros((2, 128, N), FP16)
        tsd = np.zeros((2, 128, N), FP16)
        for p in range(2):
            for i in range(2):
                h = heads[2 * p + i]
                r = 64 * i
                th32 = freqs[h][:, None].astype(np.float32) * posT  # [24, N]
                tcd[p, r : r + 24] = wrap(np.pi / 2 - th32).astype(FP16)
                tcd[p, r + 24 : r + 48] = wrap(np.pi / 2 - th32).astype(FP16)
                tsd[p, r : r + 24] = wrap(-th32).astype(FP16)
                tsd[p, r + 24 : r + 48] = wrap(th32).astype(FP16)

        def qk_pack(cols_fn, swap):
            # [384, 224]: per pack p, cols 112p.. = headA(48) 0(16) headB(48)
            w = np.zeros((IN_DIM, 224), np.float64)
            for p in range(2):
                for i in range(2):
                    h = heads[2 * p + i]
                    colblk = cols_fn(h) * sroot[h]
                    if swap:
                        colblk = np.concatenate(
                            [colblk[:, D2:], colblk[:, :D2]], axis=1
                        )
                    w[:, 112 * p + 64 * i : 112 * p + 64 * i + 48] = colblk
            return np.ascontiguousarray(w).reshape(KC, 128, 224).astype(BF16)

        q_cols = lambda h: Wq[:, h * DQ : (h + 1) * DQ].astype(np.float64)
        k_cols = lambda h: Wkv[:, h * (DQ + DV) : h * (DQ + DV) + DQ].astype(
            np.float64
        )
        wqa = qk_pack(q_cols, False)
        wqsa = qk_pack(q_cols, True)
        wka = qk_pack(k_cols, False)
        wksa = qk_pack(k_cols, True)
        wv_cols = np.concatenate(
            [Wkv[:, h * (DQ + DV) + DQ : (h + 1) * (DQ + DV)] for h in heads], axis=1
        )
        wva = np.ascontiguousarray(wv_cols).reshape(KC, 128, 192).astype(BF16)
        e2d_np = np.zeros((2, 112), np.float32)
        e2d_np[0, 0:48] = 1
        e2d_np[1, 64:112] = 1
        wo_rows = np.zeros((2, 128, 384), np.float32)
        for p in range(2):
            for i in range(2):
                h = heads[2 * p + i]
                wo_rows[p, 64 * i : 64 * i + 48] = Wout[h * DV : (h + 1) * DV, :]
        in_maps.append(
            {
                "xT": xT.astype(BF16),
                "tcd": tcd,
                "tsd": tsd,
                "wq": wqa,
                "wqs": wqsa,
                "wk": wka,
                "wks": wksa,
                "wv": wva,
                "wo": wo_rows.astype(BF16),
                "e2d": e2d_np,
            }
        )
    return in_maps


_CACHE = {}


def kernel(x, pos, Wq, Wkv, Wout, scale, _profile=False):
    from concourse.bass_utils import run_bass_kernel_spmd

    x = np.asarray(x)
    pos = np.asarray(pos)
    Wq = np.asarray(Wq)
    Wkv = np.asarray(Wkv)
    Wout = np.asarray(Wout)
    scale = np.asarray(scale)

    s0 = float(scale.reshape(-1)[0])
    assert np.allclose(scale, s0, rtol=1e-6), "non-uniform scale unsupported"
    if "nc" not in _CACHE:
        nc_new = build_nc(1.0 / s0)
        nc_new.finalize()
        _CACHE["nc"] = nc_new
    nc = _CACHE["nc"]

    in_maps = make_in_maps(x, pos, Wq, Wkv, Wout, scale)
    res = run_bass_kernel_spmd(
        nc, in_maps, core_ids=list(range(NCORES)), trace=_profile
    )
    outs = [r["out"] for r in res.results]
    full = np.zeros((B, N, IN_DIM), np.float32)
    for b in range(B):
        full[b] = outs[2 * b].astype(np.float32) + outs[2 * b + 1].astype(np.float32)
    if _profile:
        _CACHE["exec_time_ns"] = res.exec_time_ns
        _CACHE["profile_json"] = res.profile_json
    return full


# revision 41
# speedup vs baseline: 1.0444x; 1.0444x over previous
"""Trainium2 Bass kernel for nn_Attention_62706522521647.

Dense multi-head attention with QK-L2-norm (learnable scale) + axial RoPE,
B=4 N=2048 H=8 DQ=DV=48, IN_DIM=384, f32 inputs/outputs.

Sharding (8 cores, no collectives): core c handles batch b=c//2 and the
4 heads [4*(c%2), 4*(c%2)+4).  Each core computes a partial output
(its heads' contribution through the output projection); the host sums
the two partials per batch.

Per-core design (v2 — ACT-exp is the critical path, everything else hides
under it):
 - Engine APs start at partitions 0/32/64/96, so two heads are packed per
   [128, N] tile at rows 0-47 and 64-111 (pad rows zeroed via zero weight
   columns).
 - xT fed pre-transposed: xT [384, 2048] bf16 (3 chunks of [128, 2048]).
 - RoPE swap(q) via a SECOND projection with host-swapped weight columns;
   rotation is qr = raw*C2 + swp*S2 with signs baked into host-wrapped
   angles (HW Sin needs [-pi,pi]; cos = Sin(wrap(pi/2 - theta))).  Angle
   tables shipped fp16 (halves the DMA).
 - BOTH q and k are pre-normalized: rsq = exp(-0.5*ln(ssq/s + eps)) on ACT
   (ln+exp live in ONE table set with the attention exps -> only 2
   ACT_TABLE_LOADs in the whole kernel: trig + natural_log_exp).
   ssq for both q-halves packed into one psum tile via tile_position
   (0, 32*qh) so a single ln/exp pair covers the whole tensor.
 - scores TRANSPOSED: sT[k, q] = kn-chunk.T @ qn, 2 heads row-packed via
   tile_position (0,0)/(64,0); exp with immediate scale=1 (no per-partition
   operand); softmax denominator via ones column in the AV stationary
   [v|0*16|1] (M=65): Z lands in psum row 64.  No max-subtraction
   (scores in [-10,10]).
 - Attention inner loop is software-pipelined: scores(ch+1) are emitted
   BEFORE AV(ch) so the PE fills the gap while ACT streams exps; PSUM =
   2x scores bufs (4 banks) + 2x AV accumulators (4 banks).
 - Z reciprocal via DVE reciprocal_approx_fast (51 ULP, off the ACT path);
   normalize of block b is emitted inside block b+1 (hook at ch==3) so its
   PE/DVE work hides under the exp stream.  Out-projection chunks 0-7 are
   emitted inside the last block (hook at ch==10); 8-15 at the tail.
"""

import math

import numpy as np
import ml_dtypes

B, N, H, DQ, DV = 4, 2048, 8, 48, 48
IN_DIM = H * DQ  # 384
D2 = DQ // 2  # 24
MAX_FREQ = 10.0
EPS = 1e-6
NCORES = 8
HPC = 4  # heads per core
KC = IN_DIM // 128  # 3 contraction chunks for projections
NCH = N // 128  # 16 k-chunks of 128
NQH = 2  # q halves of 1024
QW = 1024  # q tile width
BF16 = ml_dtypes.bfloat16
FP16 = np.float16


def _freqs_np():
    """Reference freqs (numpy f64 ~1e-7 from the jax f32 original — far
    inside the fp16 angle quantization already applied to the tables)."""
    log_min = math.log(math.pi)
    log_max = math.log(MAX_FREQ * math.pi)
    n = H * D2
    f = np.exp(np.linspace(log_min, log_max, n + 1)[:-1])
    return f.reshape(D2, H).T.astype(np.float32)  # [H, 24]


def build_nc(inv_scale: float):
    import concourse.bass as bass
    import concourse.tile as tile
    from concourse import bacc, mybir

    from concourse.alu_op_type import AluOpType

    dt = mybir.dt
    AF = mybir.ActivationFunctionType
    F32, B16, F16 = dt.float32, dt.bfloat16, dt.float16

    nc = bacc.Bacc("TRN2")

    xT = nc.dram_tensor("xT", [KC, 128, N], B16, kind="ExternalInput")
    tcd = nc.dram_tensor("tcd", [2, 128, N], F16, kind="ExternalInput")
    tsd = nc.dram_tensor("tsd", [2, 128, N], F16, kind="ExternalInput")
    # q/k weights: per pack 112 cols (headA 0-47, zeros 48-63, headB 64-111)
    wq = nc.dram_tensor("wq", [KC, 128, 224], B16, kind="ExternalInput")
    wqs = nc.dram_tensor("wqs", [KC, 128, 224], B16, kind="ExternalInput")
    wk = nc.dram_tensor("wk", [KC, 128, 224], B16, kind="ExternalInput")
    wks = nc.dram_tensor("wks", [KC, 128, 224], B16, kind="ExternalInput")
    wv = nc.dram_tensor("wv", [KC, 128, 192], B16, kind="ExternalInput")
    wo = nc.dram_tensor("wo", [2, 128, 384], B16, kind="ExternalInput")
    e2d = nc.dram_tensor("e2d", [2, 112], F32, kind="ExternalInput")
    out = nc.dram_tensor("out", [N, IN_DIM], F16, kind="ExternalOutput")

    with tile.TileContext(nc) as tc:
        with (
            tc.tile_pool(name="consts", bufs=1) as consts,
            tc.tile_pool(name="trig", bufs=2) as trig,
            tc.tile_pool(name="qk", bufs=1) as qkpool,
            tc.tile_pool(name="work", bufs=2) as work,
            tc.tile_pool(name="esb", bufs=3) as esb,
            tc.tile_pool(name="psS", bufs=2, space=bass.MemorySpace.PSUM) as psS,
            tc.tile_pool(name="psO", bufs=2, space=bass.MemorySpace.PSUM) as psO,
        ):
            # ------------- input DMAs: merged, via HWDGE (no gpsimd) -------
            # sync ring: compute inputs in consumption order; scalar ring:
            # trig tables (feeds the Sins issued on the same ACT queue)
            xT_all = consts.tile([128, KC, N], B16, tag="xT")
            nc.sync.dma_start(out=xT_all, in_=xT.rearrange("k p n -> p k n"))
            xT_sb = [xT_all[:, kc, :] for kc in range(KC)]
            wv_all = consts.tile([128, KC, 192], B16, tag="wv")
            nc.sync.dma_start(out=wv_all, in_=wv.rearrange("k p m -> p k m"))
            wv_sb = [wv_all[:, kc, :] for kc in range(KC)]
            w_sb = {}
            for nm, hd in (("wq", wq), ("wqs", wqs), ("wk", wk), ("wks", wks)):
                t = consts.tile([128, KC, 224], B16, tag=nm, name=nm)
                nc.sync.dma_start(out=t, in_=hd.rearrange("k p m -> p k m"))
                for kc in range(KC):
                    w_sb[(nm, kc)] = t[:, kc, :]
            wo_all = consts.tile([128, 2, 384], B16, tag="wo")
            nc.sync.dma_start(out=wo_all, in_=wo.rearrange("k p m -> p k m"))
            wo_sb = [wo_all[:, p, :] for p in range(2)]
            tc_all = consts.tile([128, 2, N], F16, tag="tc")
            nc.scalar.dma_start(out=tc_all, in_=tcd.rearrange("k p n -> p k n"))
            ts_all = consts.tile([128, 2, N], F16, tag="ts")
            nc.scalar.dma_start(out=ts_all, in_=tsd.rearrange("k p n -> p k n"))
            th_sb = {p: (tc_all[:, p, :], ts_all[:, p, :]) for p in range(2)}
            # E2 replicated at rows 0-1 and 32-33 (matmul requires lhsT and
            # rhs at the same base partition; rsq for qh=1 sits at rows 32-33)
            E2 = consts.tile([34, 112], F32, tag="E2")
            nc.scalar.dma_start(out=E2[0:2, :], in_=e2d[:])
            nc.scalar.dma_start(out=E2[32:34, :], in_=e2d[:])

            # ---------------- constants / memsets (gpsimd: free engine) ---
            # ssq reduction stationaries, M=34 with zero pad cols so every
            # psum row of ps_ssq is written (race-detector/garbage safety):
            # qh0 sums land in rows 0-1 (ones2a), qh1 in rows 32-33 (ones2b)
            ones2a = consts.tile([128, 128], B16, tag="ones2a")
            nc.vector.memset(ones2a, 0.0)
            nc.vector.memset(ones2a[0:48, 0:1], 1.0)
            nc.vector.memset(ones2a[64:112, 1:2], 1.0)
            ones2b = consts.tile([128, 128], B16, tag="ones2b")
            nc.vector.memset(ones2b, 0.0)
            nc.vector.memset(ones2b[0:48, 32:33], 1.0)
            nc.vector.memset(ones2b[64:112, 33:34], 1.0)
            # Ez: z-broadcast stationary: row 64 -> cols 0-47, row 96 -> 64-111
            Ez = consts.tile([128, 112], F32, tag="Ez")
            nc.vector.memset(Ez, 0.0)
            nc.vector.memset(Ez[64:65, 0:48], 1.0)
            nc.vector.memset(Ez[96:97, 64:112], 1.0)
            # zq: Z staging, all-finite by construction (rows 64/96 overwritten)
            zq = work.tile([128, QW], F32, tag="zq", bufs=1)
            nc.gpsimd.memset(zq, 1.0)
            # activation-bias constants
            cdb = consts.tile([128, 2], F32, tag="cdb")
            for col, val in enumerate([0.0, EPS]):
                nc.vector.memset(cdb[:, col : col + 1], val)
                nc.const_aps.aps[(F32, val)] = cdb[:, col : col + 1]

            # v stationary per (chunk, head): [v(48) | zeros | one].  The
            # ones column sits at 64 for slot-0 heads and 96 for slot-1
            # heads, so Z lands at psum partition 64/96 respectively and the
            # 1/Z reciprocal into zq needs no partition shift (HW custom-DVE
            # ops don't shift partitions even though the sim models it).
            v4 = consts.tile([128, NCH, HPC, 97], B16, tag="v4")
            nc.gpsimd.memset(v4[:, :, :, 48:97], 0.0)
            for hp in range(HPC):
                zc = 64 if hp % 2 == 0 else 96
                nc.gpsimd.memset(v4[:, :, hp, zc : zc + 1], 1.0)

            # packed attention outputs (pad rows must be finite zeros for the
            # out-projection: garbage bf16 could be NaN and NaN*0 = NaN)
            on_pack = [
                qkpool.tile([128, N], B16, tag=f"on{p}", name=f"on{p}")
                for p in range(2)
            ]
            for p in range(2):
                # rows 48-63 / 112-127 must be finite zeros; bases limited to
                # 32-multiples, the extra rows are overwritten by normalize
                nc.gpsimd.memset(on_pack[p][32:64, :], 0.0)
                nc.gpsimd.memset(on_pack[p][96:128, :], 0.0)

            # ---------------- trig tables: Sin first (trig table set) ------
            C2, S2 = [], []
            for p in range(2):
                tct, tst = th_sb[p]
                c2t = trig.tile([128, N], B16, tag="c2t", name=f"c2t{p}", bufs=2)
                nc.scalar.activation(c2t, tct, AF.Sin)
                C2.append(c2t)
                s2t = trig.tile([128, N], B16, tag="s2t", name=f"s2t{p}", bufs=2)
                nc.scalar.activation(s2t, tst, AF.Sin)
                S2.append(s2t)
            dummy = work.tile([128, 1], F32, tag="dummy", bufs=1)

            def pe_warmup(n):
                # HAM un-throttles (K=4/8 -> 8/8, 1.2 -> 2.4 GHz) only after
                # ~3.4us of GAPLESS PE activity; dependency-free back-to-back
                # dummy matmuls provide it.  FIFO position controls WHEN.
                wps = psS.tile([128, 512], F32, tag="s", name="warm")
                for _ in range(n):
                    nc.tensor.matmul(
                        wps, xT_sb[0][:, 0:128], xT_sb[1][:, 0:512],
                        start=True, stop=True,
                    )

            # ---------------- v projection ----------------
            pe_warmup(9)
            for ch in range(NCH):
                pool, tg = (psS, "s") if ch % 2 == 0 else (psO, "o")
                ps_v = pool.tile([128, 192], F32, tag=tg, name="ps_v")
                for kc in range(KC):
                    nc.tensor.matmul(
                        ps_v,
                        xT_sb[kc][:, 128 * ch : 128 * (ch + 1)],
                        wv_sb[kc],
                        start=(kc == 0),
                        stop=(kc == KC - 1),
                    )
                nc.scalar.copy(
                    v4[:, ch, :, 0:48],
                    ps_v.rearrange("p (h d) -> p h d", h=HPC),
                )

            # ---------------- q/k projections + rope (PE + DVE) -----------
            qn = [
                qkpool.tile([128, N], B16, tag=f"qn{p}", name=f"qn{p}")
                for p in range(2)
            ]
            kn = [
                qkpool.tile([128, N], B16, tag=f"kn{p}", name=f"kn{p}")
                for p in range(2)
            ]
            qr_t = {}  # (p, name) -> rope'd (un-normalized) [128, N] bf16
            tensors = [(p, name) for p in range(2) for name in ("q", "k")]

            for p, name in tensors:
                c2t, s2t = C2[p], S2[p]
                qr = work.tile(
                    [128, N], B16, tag="qr", name=f"qr_{name}{p}", bufs=4
                )
                qr_t[(p, name)] = qr
                for nh in range(2):
                    ns = 1024 * nh
                    raw = psS.tile([112, 1024], F32, tag="s", name="raw")
                    swp = psO.tile([112, 1024], F32, tag="o", name="swp")
                    for half in range(2):
                        hs = 512 * half
                        for kc in range(KC):
                            nc.tensor.matmul(
                                raw[:, hs : hs + 512],
                                w_sb[("w" + name, kc)][:, 112 * p : 112 * (p + 1)],
                                xT_sb[kc][:, ns + hs : ns + hs + 512],
                                start=(kc == 0),
                                stop=(kc == KC - 1),
                            )
                        for kc in range(KC):
                            nc.tensor.matmul(
                                swp[:, hs : hs + 512],
                                w_sb[("w" + name + "s", kc)][:, 112 * p : 112 * (p + 1)],
                                xT_sb[kc][:, ns + hs : ns + hs + 512],
                                start=(kc == 0),
                                stop=(kc == KC - 1),
                            )
                    t1 = work.tile([112, 1024], B16, tag="t1", name="t1", bufs=2)
                    nc.vector.tensor_mul(t1, raw, c2t[0:112, ns : ns + 1024])
                    t2 = work.tile([112, 1024], B16, tag="t2", name="t2", bufs=2)
                    nc.vector.tensor_mul(t2, swp, s2t[0:112, ns : ns + 1024])
                    # the add runs on gpsimd (idle engine) to shorten the
                    # DVE prep pole; ~2.2us each but fully overlapped
                    nc.gpsimd.tensor_tensor(
                        qr[0:112, ns : ns + 1024], t1, t2, AluOpType.add
                    )

            # norms, wave-pipelined across the 4 tensors: ssq packed [34, QW]
            # (qh0 sums rows 0-1, qh1 rows 32-33); sqrt on ACT (single table
            # set), 1/sqrt via DVE reciprocal_approx_fast (keeps the ACT
            # stream to 3 table loads total: trig -> sqrt -> exp)
            sq_t, ssq_t, rsq_t = {}, {}, {}
            for key in tensors:
                qr = qr_t[key]
                sq = work.tile([112, N], B16, tag="sq", name="sq", bufs=2)
                nc.scalar.activation(sq, qr[0:112, :], AF.Square)
                sq_t[key] = sq
            for key in tensors:
                sq = sq_t[key]
                ps_ssq = psO.tile([128, QW], F32, tag="o", name="ps_ssq")
                for hh in range(2):
                    for qh in range(NQH):
                        ns = QW * qh + 512 * hh
                        nc.tensor.matmul(
                            ps_ssq[:, 512 * hh : 512 * (hh + 1)],
                            (ones2a if qh == 0 else ones2b)[0:112, :],
                            sq[:, ns : ns + 512],
                            start=(qh == 0),
                            stop=(qh == 1),
                        )
                ssq_t[key] = ps_ssq
            for key in tensors:
                sqq = work.tile([128, QW], F32, tag="sqq", name="sqq", bufs=2)
                nc.scalar.activation(
                    sqq, ssq_t[key], AF.Sqrt, scale=inv_scale, bias=EPS
                )
                rsq = work.tile([128, QW], F32, tag="rsq", name="rsq", bufs=4)
                nc.vector.reciprocal_approx_fast(out=rsq, in_=sqq)
                rsq_t[key] = rsq
            # prefetch the exp table set now (off the attention critical path)
            nc.scalar.activation(dummy, cdb[:, 0:1], AF.Exp)
            for i, (p, name) in enumerate(tensors):
                qr, rsq = qr_t[(p, name)], rsq_t[(p, name)]
                dst = qn[p] if name == "q" else kn[p]
                for qh in range(NQH):
                    pool, tg = ((psS, "s") if (2 * i + qh) % 2 == 0
                                else (psO, "o"))
                    ps_rb = pool.tile([112, QW], F32, tag=tg, name="ps_rb")
                    for hh in range(2):
                        nc.tensor.matmul(
                            ps_rb[:, 512 * hh : 512 * (hh + 1)],
                            E2[32 * qh : 32 * qh + 2, :],
                            rsq[32 * qh : 32 * qh + 2,
                                512 * hh : 512 * (hh + 1)],
                            start=True,
                            stop=True,
                        )
                    qs = QW * qh
                    nc.vector.tensor_mul(
                        dst[0:112, qs : qs + QW],
                        qr[0:112, qs : qs + QW],
                        ps_rb,
                    )

            # ---------------- attention ----------------
            row0 = {0: 0, 1: 64}  # head slot -> pack row offset

            def normalize_a(o0, o1, tail=False):
                """Block-end stage: free the AV psum banks ASAP.  1/Z via
                reciprocal_approx_fast straight off the Z rows of o_ps into
                zq rows 64/96 (zq is memset 1.0, so every row stays finite
                for the Ez broadcast); o values copied out to bf16."""
                cp = nc.scalar.copy if tail else nc.vector.tensor_copy
                obufs = []
                for i, o in enumerate((o0, o1)):
                    zr = 64 + 32 * i
                    cp(zq[zr : zr + 1, :], o[zr : zr + 1, :])
                    ob = work.tile([48, QW], B16, tag=f"ob{i}", name=f"ob{i}",
                                   bufs=2)
                    cp(ob, o[0:48, :])
                    obufs.append(ob)
                # full-tile SBUF->SBUF recip (the only form HW handles)
                rzb = work.tile([128, QW], F32, tag="rzb", name="rzb", bufs=2)
                nc.vector.reciprocal_approx_fast(out=rzb, in_=zq)
                return obufs, rzb

            def normalize_b(p, qh, obufs, rzb):
                """Next-block stage (PE + DVE, hidden under the exp stream):
                broadcast 1/Z over the head rows and scale."""
                qs = QW * qh
                ps_r = psS.tile([112, QW], F32, tag="s", name="ps_r")
                for hh in range(2):
                    nc.tensor.matmul(
                        ps_r[:, 512 * hh : 512 * (hh + 1)],
                        Ez,
                        rzb[:, 512 * hh : 512 * (hh + 1)],
                        start=True,
                        stop=True,
                    )
                for i in range(2):
                    r = row0[i]
                    nc.vector.tensor_mul(
                        on_pack[p][r : r + 48, qs : qs + QW],
                        obufs[i],
                        ps_r[r : r + 48, :],
                    )

            # output staged in SBUF, shipped in two big HWDGE DMAs
            osb_all = consts.tile([128, NCH, 384], F16, tag="osb")
            out_r = out.rearrange("(c p) m -> p c m", p=128)

            def outproj(chs, flush, tail=False):
                cp = nc.scalar.copy if tail else nc.vector.tensor_copy
                for ch in chs:
                    ns = 128 * ch
                    ps_out = psS.tile([128, 384], F32, tag="s", name="ps_out")
                    for p in range(2):
                        nc.tensor.matmul(
                            ps_out,
                            on_pack[p][:, ns : ns + 128],
                            wo_sb[p],
                            start=(p == 0),
                            stop=(p == 1),
                        )
                    cp(osb_all[:, ch, :], ps_out)
                nc.sync.dma_start(
                    out=out_r[:, flush[0] : flush[1], :],
                    in_=osb_all[:, flush[0] : flush[1], :],
                )

            def attn_block(p, qh, hooks, warm=0):
                qs = QW * qh
                o = [
                    psO.tile([65 + 32 * i, QW], F32, tag="o", name=f"o{i}")
                    for i in range(2)
                ]
                stiles = {}

                def emit_scores(ch):
                    ks = 128 * ch
                    for i in range(2):
                        r = row0[i]
                        s = psS.tile([128, QW], F32, tag="s", name=f"s{i}")
                        for hh in range(2):
                            nc.tensor.matmul(
                                s[:, 512 * hh : 512 * (hh + 1)],
                                kn[p][r : r + 48, ks : ks + 128],
                                qn[p][r : r + 48,
                                      qs + 512 * hh : qs + 512 * (hh + 1)],
                                start=True,
                                stop=True,
                                tile_position=(r, 0),
                            )
                        stiles[(ch, i)] = s

                if warm:
                    # FIFO slot right before the first scores: executes after
                    # the last prep matmul, flipping HAM to 8/8 for the
                    # attention stream (gap to first scores < 3.4us MID window)
                    pe_warmup(warm)
                emit_scores(0)
                for ch in range(NCH):
                    es = []
                    for i in range(2):
                        e = esb.tile([128, QW], B16, tag=f"e{i}", name=f"e{i}")
                        nc.scalar.activation(e, stiles.pop((ch, i)), AF.Exp)
                        es.append(e)
                    if ch + 1 < NCH:
                        emit_scores(ch + 1)
                    for i in range(2):
                        for hh in range(2):
                            nc.tensor.matmul(
                                o[i][:, 512 * hh : 512 * (hh + 1)],
                                v4[:, ch, 2 * p + i, 0 : 65 + 32 * i],
                                es[i][:, 512 * hh : 512 * (hh + 1)],
                                start=(ch == 0),
                                stop=(ch == NCH - 1),
                            )
                    if ch in (5, 11):
                        # periodic re-warm: HAM oscillates under the micro-
                        # idle pattern of the exp-bound stream (K18 failure
                        # mode); short gapless bursts keep K=8/8
                        pe_warmup(6)
                    if ch in hooks:
                        hooks[ch]()
                return o

            prev = None
            for p in range(2):
                for qh in range(NQH):
                    hooks = {}
                    if prev is not None:
                        pv = prev
                        hooks[3] = lambda pv=pv: normalize_b(*pv)
                    if (p, qh) == (1, 1):
                        hooks[10] = lambda: outproj(range(8), (0, 8))
                    o = attn_block(p, qh, hooks, warm=9 if prev is None else 0)
                    obufs, rzb = normalize_a(o[0], o[1], tail=(p, qh) == (1, 1))
                    prev = (p, qh, obufs, rzb)
            normalize_b(*prev)
            outproj(range(8, 16), (8, 16), tail=True)

    return nc


def make_in_maps(x, pos, Wq, Wkv, Wout, scale):
    """Build the 8 per-core input dicts (host-side sharding + layout)."""
    freqs = _freqs_np()  # [H, 24]
    sroot = np.sqrt(scale.astype(np.float64))  # [H]
    in_maps = []
    for c in range(NCORES):
        b = c // 2
        hb = HPC * (c % 2)
        heads = list(range(hb, hb + HPC))
        xb = x[b].astype(np.float32)  # [N, 384]
        xT = np.ascontiguousarray(xb.T).reshape(KC, 128, N)
        posT = np.ascontiguousarray(pos[b].T).astype(np.float32)  # [24, N]

        def wrap(a):  # -> [-pi, pi], in f64 then back
            return (np.mod(a.astype(np.float64) + np.pi, 2 * np.pi) - np.pi)

        tcd = np.zeros((2, 128, N), FP16)
        tsd = np.zeros((2, 128, N), FP16)
        for p in range(2):
            for i in range(2):
                h = heads[2 * p + i]
                r = 64 * i
                th32 = freqs[h][:, None].astype(np.float32) * posT  # [24, N]
                tcd[p, r : r + 24] = wrap(np.pi / 2 - th32).astype(FP16)
                tcd[p, r + 24 : r + 48] = wrap(np.pi / 2 - th32).astype(FP16)
                tsd[p, r : r + 24] = wrap(-th32).astype(FP16)
                tsd[p, r + 24 : r + 48] = wrap(th32).astype(FP16)

        def qk_pack(cols_fn, swap):
            # [384, 224]: per pack p, cols 112p.. = headA(48) 0(16) headB(48)
            w = np.zeros((IN_DIM, 224), np.float64)
            for p in range(2):
                for i in range(2):
                    h = heads[2 * p + i]
                    colblk = cols_fn(h) * sroot[h]
                    if swap:
                        colblk = np.concatenate(
                            [colblk[:, D2:], colblk[:, :D2]], axis=1
                        )
                    w[:, 112 * p + 64 * i : 112 * p + 64 * i + 48] = colblk
            return np.ascontiguousarray(w).reshape(KC, 128, 224).astype(BF16)

        q_cols = lambda h: Wq[:, h * DQ : (h + 1) * DQ].astype(np.float64)
        k_cols = lambda h: Wkv[:, h * (DQ + DV) : h * (DQ + DV) + DQ].astype(
            np.float64
        )
        wqa = qk_pack(q_cols, False)
        wqsa = qk_pack(q_cols, True)
        wka = qk_pack(k_cols, False)
        wksa = qk_pack(k_cols, True)
        wv_cols = np.concatenate(
            [Wkv[:, h * (DQ + DV) + DQ : (h + 1) * (DQ + DV)] for h in heads], axis=1
        )
        wva = np.ascontiguousarray(wv_cols).reshape(KC, 128, 192).astype(BF16)
        e2d_np = np.zeros((2, 112), np.float32)
        e2d_np[0, 0:48] = 1
        e2d_np[1, 64:112] = 1
        wo_rows = np.zeros((2, 128, 384), np.float32)
        for p in range(2):
            for i in range(2):
                h = heads[2 * p + i]
                wo_rows[p, 64 * i : 64 * i + 48] = Wout[h * DV : (h + 1) * DV, :]
        in_maps.append(
            {
                "xT": xT.astype(BF16),
                "tcd": tcd,
                "tsd": tsd,
                "wq": wqa,
                "wqs": wqsa,
                "wk": wka,
                "wks": wksa,
                "wv": wva,
                "wo": wo_rows.astype(BF16),
                "e2d": e2d_np,
            }
        )
    return in_maps


_CACHE = {}


def kernel(x, pos, Wq, Wkv, Wout, scale, _profile=False):
    from concourse.bass_utils import run_bass_kernel_spmd

    x = np.asarray(x)
    pos = np.asarray(pos)
    Wq = np.asarray(Wq)
    Wkv = np.asarray(Wkv)
    Wout = np.asarray(Wout)
    scale = np.asarray(scale)

    s0 = float(scale.reshape(-1)[0])
    assert np.allclose(scale, s0, rtol=1e-6), "non-uniform scale unsupported"
    if "nc" not in _CACHE:
        nc_new = build_nc(1.0 / s0)
        nc_new.finalize()
        _CACHE["nc"] = nc_new
    nc = _CACHE["nc"]

    in_maps = make_in_maps(x, pos, Wq, Wkv, Wout, scale)
    res = run_bass_kernel_spmd(
        nc, in_maps, core_ids=list(range(NCORES)), trace=_profile
    )
    outs = [r["out"] for r in res.results]
    full = np.zeros((B, N, IN_DIM), np.float32)
    for b in range(B):
        full[b] = outs[2 * b].astype(np.float32) + outs[2 * b + 1].astype(np.float32)
    if _profile:
        _CACHE["exec_time_ns"] = res.exec_time_ns
        _CACHE["profile_json"] = res.profile_json
    return full
